# revision 5
# baseline (speedup 1.0000x reference)
"""Negative-sampling linear kernel for Trainium2 (8 NeuronCores, SPMD).

Reference computation:
    pos = W[y]; neg = W[neg_idx]
    out = concat([einsum('bd,bd->b', x, pos)[:, None],
                  einsum('bd,bkd->bk', x, neg)], axis=1)      # [B, 1+K]

Strategy: data-parallel over batch (128 samples per core, one per SBUF
partition); full W replicated in HBM. The gather of 513 W-rows per
sample is split into:

  * a dense part — the vocab is cut into 4 quarters of 25000 rows so
    row indices fit int16, and per quarter each partition's first NB
    hits are packed into fixed [128, NB] rectangles gathered with bulk
    `dma_gather` (one instruction per 31-column chunk, ~0.3ns/descriptor
    SWDGE emission);
  * a ragged tail — slots beyond NB per quarter (mixed quarters, int32
    indices) gathered one column at a time with `indirect_dma_start`.

Rectangles are padded with index 0 (gathered then dropped). After each
gathered chunk lands, one DVE multiply (x broadcast across columns) and
a DVE/ACT reduction produce the per-sample dot products. The host
builds the packing and inverse permutation, and un-permutes the device
output; tail overflow beyond TT columns (never for the reference input
sizes/seed) falls back to a host-side dot for those few slots.
"""

import numpy as np

B, D, V, K = 1024, 512, 100000, 512
NCORES = 8
BLOC = B // NCORES          # 128 == SBUF partitions
NK = K + 1                  # 513 logits per sample
NQ = 4                      # vocab quarters (int16-addressable)
VQ = V // NQ                # 25000 rows per quarter
NB = 124                    # dense slots per quarter per partition
NBC = 31                    # dense chunk columns (124 = 4 * 31)
NDC = NB // NBC             # dense chunks per quarter
TT = 48                     # tail columns
TC = 24                     # tail chunk columns
NSLOT = NQ * NB + TT        # 544 device columns
DGI_PER_CHUNK = NBC * BLOC // 16   # int16s per partition row per chunk (248)
ACT_COLS = 8                # reduce columns per chunk offloaded to ScalarE
_PROG = None


def build_program(act_cols=ACT_COLS, gbufs=2):
    import concourse.bacc as bacc
    import concourse.bass as bass
    import concourse.mybir as mybir
    import concourse.tile as tile

    f32 = mybir.dt.float32
    i32 = mybir.dt.int32
    i16 = mybir.dt.int16

    nc = bacc.Bacc("TRN2")
    w_t = nc.dram_tensor("W", [V, D], f32, kind="ExternalInput")
    x_t = nc.dram_tensor("x", [BLOC, D], f32, kind="ExternalInput")
    dgi_t = nc.dram_tensor(
        "dgi", [BLOC, NQ * NDC * DGI_PER_CHUNK], i16, kind="ExternalInput"
    )
    tidx_t = nc.dram_tensor("tidx", [BLOC, TT], i32, kind="ExternalInput")
    out_t = nc.dram_tensor("out", [BLOC, NSLOT], f32, kind="ExternalOutput")

    with tile.TileContext(nc) as tc:
        with (
            tc.tile_pool(name="const", bufs=1) as cpool,
            tc.tile_pool(name="g", bufs=gbufs) as gpool,
        ):
            xt = cpool.tile([BLOC, D], f32, tag="x")
            nc.sync.dma_start(out=xt[:], in_=x_t[:])
            dgi = cpool.tile([BLOC, NQ * NDC * DGI_PER_CHUNK], i16, tag="dgi")
            nc.sync.dma_start(out=dgi[:], in_=dgi_t[:])
            tit = cpool.tile([BLOC, TT], i32, tag="tidx")
            nc.sync.dma_start(out=tit[:], in_=tidx_t[:])
            ot = cpool.tile([BLOC, NSLOT], f32, tag="out")

            def compute(g, ncols, col0):
                """multiply gathered rows by x and reduce each column."""
                x_b = (
                    xt[:]
                    .rearrange("p (u d) -> p u d", u=1)
                    .to_broadcast([BLOC, ncols, D])
                )
                nc.vector.tensor_tensor(
                    out=g[:], in0=g[:], in1=x_b, op=mybir.AluOpType.mult
                )
                ndve = ncols - act_cols
                if ndve > 0:
                    nc.vector.tensor_reduce(
                        out=ot[:, col0 : col0 + ndve],
                        in_=g[:, :ndve, :],
                        op=mybir.AluOpType.add,
                        axis=mybir.AxisListType.X,
                    )
                for j in range(max(ndve, 0), ncols):
                    nc.scalar.activation(
                        out=g[:, j, :],
                        in_=g[:, j, :],
                        func=mybir.ActivationFunctionType.Copy,
                        accum_out=ot[:, col0 + j : col0 + j + 1],
                    )

            # dense quarters via bulk dma_gather
            for qq in range(NQ):
                w_q = w_t[qq * VQ : (qq + 1) * VQ, :]
                for cc in range(NDC):
                    ci = qq * NDC + cc
                    g = gpool.tile([BLOC, NBC, D], f32, tag="g")
                    nc.gpsimd.dma_gather(
                        out_ap=g[:],
                        in_ap=w_q,
                        idxs_ap=dgi[
                            :, ci * DGI_PER_CHUNK : (ci + 1) * DGI_PER_CHUNK
                        ],
                        num_idxs=NBC * BLOC,
                        num_idxs_reg=NBC * BLOC,
                        elem_size=D,
                        single_packet=False,
                    )
                    compute(g, NBC, qq * NB + cc * NBC)

            # ragged tail via per-column indirect gathers
            for tc_i in range(TT // TC):
                g = gpool.tile([BLOC, TC, D], f32, tag="g")
                for j in range(TC):
                    col = tc_i * TC + j
                    nc.gpsimd.indirect_dma_start(
                        out=g[:, j, :],
                        out_offset=None,
                        in_=w_t[:],
                        in_offset=bass.IndirectOffsetOnAxis(
                            ap=tit[:, col : col + 1], axis=0
                        ),
                    )
                compute(g, TC, NQ * NB + tc_i * TC)

            nc.sync.dma_start(out=out_t[:], in_=ot[:])

    nc.compile()
    return nc


def _prepare(idx):
    """Pack indices into dense quarter rectangles + ragged tail.

    idx: [B, NK] int64/int32 global rows.
    Returns (dgi [B, NQ*NDC*DGI_PER_CHUNK] i16, tidx [B, TT] i32,
             perm [B, NK] intp  (device column of each original slot),
             overflow list of (b, k) pairs that did not fit).
    """
    idx = np.asarray(idx)
    q = (idx // VQ).astype(np.int64)                      # [B, NK]
    order = np.argsort(q, axis=1, kind="stable")          # [B, NK]
    qs = np.take_along_axis(q, order, 1)
    gs = np.take_along_axis(idx, order, 1)
    cnt = (q[:, :, None] == np.arange(NQ)[None, None, :]).sum(1)   # [B, NQ]
    start = np.zeros((B, NQ), np.int64)
    start[:, 1:] = np.cumsum(cnt, 1)[:, :-1]
    s = np.arange(NK)[None, :]
    j = s - np.take_along_axis(start, qs, 1)              # within-quarter rank
    dense_mask = j < NB
    excess = np.maximum(cnt - NB, 0)
    exb = np.zeros((B, NQ), np.int64)
    exb[:, 1:] = np.cumsum(excess, 1)[:, :-1]
    tpos = np.take_along_axis(exb, qs, 1) + (j - NB)
    devcol = np.where(dense_mask, qs * NB + j, NQ * NB + tpos)

    pi = np.broadcast_to(np.arange(B)[:, None], (B, NK))
    dense_local = np.zeros((B, NQ, NB), np.int16)
    sel = dense_mask
    dense_local[pi[sel], qs[sel], j[sel]] = (gs - qs * VQ)[sel].astype(np.int16)
    tail = np.zeros((B, TT), np.int32)
    sel2 = (~dense_mask) & (tpos < TT)
    tail[pi[sel2], tpos[sel2]] = gs[sel2].astype(np.int32)

    over = (~dense_mask) & (tpos >= TT)
    overflow = []
    if over.any():
        ob, os_ = np.nonzero(over)
        overflow = list(zip(ob.tolist(), order[ob, os_].tolist()))
        devcol[ob, os_] = 0  # arbitrary; fixed up on host

    perm = np.zeros((B, NK), np.intp)
    np.put_along_axis(perm, order, devcol, 1)

    # wrap dense chunks into the dma_gather int16 layout:
    # list position i*128+p -> (partition (i*128+p) % 16, slot // 16),
    # replicated across the 8 groups of 16 partitions.
    dgi = np.zeros((B, NQ * NDC * DGI_PER_CHUNK), np.int16)
    for c in range(NCORES):
        dl = dense_local[c * BLOC : (c + 1) * BLOC]       # [128, NQ, NB]
        for qq in range(NQ):
            for cc in range(NDC):
                ci = qq * NDC + cc
                vals = dl[:, qq, cc * NBC : (cc + 1) * NBC]   # [128 p, NBC i]
                a = vals.T.reshape(NBC, 8, 16)                # [i, phi, plo]
                w = a.transpose(2, 0, 1).reshape(16, DGI_PER_CHUNK)
                dgi[
                    c * BLOC : (c + 1) * BLOC,
                    ci * DGI_PER_CHUNK : (ci + 1) * DGI_PER_CHUNK,
                ] = np.tile(w, (8, 1))
    return dgi, tail, perm, overflow


def kernel(x, y, neg_idx, W):
    global _PROG
    from concourse.bass_utils import run_bass_kernel_spmd

    x = np.ascontiguousarray(np.asarray(x), dtype=np.float32)
    W = np.ascontiguousarray(np.asarray(W), dtype=np.float32)
    idx = np.concatenate(
        [np.asarray(y).reshape(B, 1), np.asarray(neg_idx).reshape(B, K)], axis=1
    ).astype(np.int64)

    dgi, tail, perm, overflow = _prepare(idx)

    if _PROG is None:
        _PROG = build_program()

    in_maps = [
        {
            "W": W,
            "x": x[i * BLOC : (i + 1) * BLOC],
            "dgi": dgi[i * BLOC : (i + 1) * BLOC],
            "tidx": tail[i * BLOC : (i + 1) * BLOC],
        }
        for i in range(NCORES)
    ]
    res = run_bass_kernel_spmd(_PROG, in_maps, list(range(NCORES)))
    dev = np.concatenate([res.results[i]["out"] for i in range(NCORES)], axis=0)
    out = np.take_along_axis(dev, perm, 1).astype(np.float32)
    for b, k in overflow:
        out[b, k] = float(np.dot(x[b], W[idx[b, k]]))
    return out


# revision 6
# speedup vs baseline: 1.6314x; 1.6314x over previous
"""Negative-sampling linear kernel for Trainium2 (8 NeuronCores, SPMD).

Reference computation:
    pos = W[y]; neg = W[neg_idx]
    out = concat([einsum('bd,bd->b', x, pos)[:, None],
                  einsum('bd,bkd->bk', x, neg)], axis=1)      # [B, 1+K]

Strategy: data-parallel over batch (128 samples per core, one per SBUF
partition); full W replicated in HBM. The gather of 513 W-rows per
sample is split into:

  * a dense part — the vocab is cut into 4 quarters of 25000 rows so
    row indices fit int16, and per quarter each partition's first NB
    hits are packed into fixed [128, NB] rectangles gathered with bulk
    `dma_gather` (one instruction per 31-column chunk, ~0.3ns/descriptor
    SWDGE emission);
  * a ragged tail — slots beyond NB per quarter (mixed quarters, int32
    indices) gathered one column at a time with `indirect_dma_start`.

Rectangles are padded with index 0 (gathered then dropped). After each
gathered chunk lands, one DVE multiply (x broadcast across columns) and
a DVE/ACT reduction produce the per-sample dot products. The host
builds the packing and inverse permutation, and un-permutes the device
output; tail overflow beyond TT columns (never for the reference input
sizes/seed) falls back to a host-side dot for those few slots.
"""

import numpy as np

B, D, V, K = 1024, 512, 100000, 512
NCORES = 8
BLOC = B // NCORES          # 128 == SBUF partitions
NK = K + 1                  # 513 logits per sample
NQ = 4                      # vocab quarters (int16-addressable)
VQ = V // NQ                # 25000 rows per quarter
NB = 112                    # dense slots per quarter per partition
NBC = 16                    # dense chunk columns (112 = 7 * 16)
NDC = NB // NBC             # dense chunks per quarter
TT = 84                     # tail columns
TC = 14                     # tail chunk columns
NSLOT = NQ * NB + TT        # 532 device columns
DGI_PER_CHUNK = NBC * BLOC // 16   # int16s per partition row per chunk (128)
ACT_COLS = 10               # reduce columns per chunk offloaded to ScalarE
_PROG = None


def build_program(act_cols=ACT_COLS, gbufs=4):
    import concourse.bacc as bacc
    import concourse.bass as bass
    import concourse.mybir as mybir
    import concourse.tile as tile

    f32 = mybir.dt.float32
    i32 = mybir.dt.int32
    i16 = mybir.dt.int16

    nc = bacc.Bacc("TRN2")
    w_t = nc.dram_tensor("W", [V, D], f32, kind="ExternalInput")
    x_t = nc.dram_tensor("x", [BLOC, D], f32, kind="ExternalInput")
    dgi_t = nc.dram_tensor(
        "dgi", [BLOC, NQ * NDC * DGI_PER_CHUNK], i16, kind="ExternalInput"
    )
    tidx_t = nc.dram_tensor("tidx", [BLOC, TT], i32, kind="ExternalInput")
    out_t = nc.dram_tensor("out", [BLOC, NSLOT], f32, kind="ExternalOutput")

    with tile.TileContext(nc) as tc:
        with (
            tc.tile_pool(name="const", bufs=1) as cpool,
            tc.tile_pool(name="g", bufs=gbufs) as gpool,
        ):
            xt = cpool.tile([BLOC, D], f32, tag="x")
            nc.sync.dma_start(out=xt[:], in_=x_t[:])
            dgi = cpool.tile([BLOC, NQ * NDC * DGI_PER_CHUNK], i16, tag="dgi")
            nc.sync.dma_start(out=dgi[:], in_=dgi_t[:])
            tit = cpool.tile([BLOC, TT], i32, tag="tidx")
            nc.sync.dma_start(out=tit[:], in_=tidx_t[:])
            ot = cpool.tile([BLOC, NSLOT], f32, tag="out")

            def bcast_x(ncols):
                return (
                    xt[:]
                    .rearrange("p (u d) -> p u d", u=1)
                    .to_broadcast([BLOC, ncols, D])
                )

            def compute(g, ncols, col0):
                """multiply gathered rows by x and reduce each column.

                The ScalarE share of the columns is multiplied (and its
                per-column accum-reductions issued) first, so ScalarE can
                start while VectorE multiplies the rest."""
                ac = min(act_cols, ncols)
                ndve = ncols - ac
                if ac > 0 and ndve > 0:
                    nc.vector.tensor_tensor(
                        out=g[:, ndve:, :], in0=g[:, ndve:, :],
                        in1=bcast_x(ac), op=mybir.AluOpType.mult,
                    )
                    for j in range(ndve, ncols):
                        nc.scalar.activation(
                            out=g[:, j, :], in_=g[:, j, :],
                            func=mybir.ActivationFunctionType.Copy,
                            accum_out=ot[:, col0 + j : col0 + j + 1],
                        )
                    nc.vector.tensor_tensor(
                        out=g[:, :ndve, :], in0=g[:, :ndve, :],
                        in1=bcast_x(ndve), op=mybir.AluOpType.mult,
                    )
                    nc.vector.tensor_reduce(
                        out=ot[:, col0 : col0 + ndve], in_=g[:, :ndve, :],
                        op=mybir.AluOpType.add, axis=mybir.AxisListType.X,
                    )
                else:
                    nc.vector.tensor_tensor(
                        out=g[:], in0=g[:], in1=bcast_x(ncols),
                        op=mybir.AluOpType.mult,
                    )
                    if ndve > 0:
                        nc.vector.tensor_reduce(
                            out=ot[:, col0 : col0 + ndve], in_=g[:, :ndve, :],
                            op=mybir.AluOpType.add, axis=mybir.AxisListType.X,
                        )
                    for j in range(max(ndve, 0), ncols):
                        nc.scalar.activation(
                            out=g[:, j, :], in_=g[:, j, :],
                            func=mybir.ActivationFunctionType.Copy,
                            accum_out=ot[:, col0 + j : col0 + j + 1],
                        )

            # dense quarters via bulk dma_gather
            for qq in range(NQ):
                w_q = w_t[qq * VQ : (qq + 1) * VQ, :]
                for cc in range(NDC):
                    ci = qq * NDC + cc
                    g = gpool.tile([BLOC, NBC, D], f32, tag="g")
                    nc.gpsimd.dma_gather(
                        out_ap=g[:],
                        in_ap=w_q,
                        idxs_ap=dgi[
                            :, ci * DGI_PER_CHUNK : (ci + 1) * DGI_PER_CHUNK
                        ],
                        num_idxs=NBC * BLOC,
                        num_idxs_reg=NBC * BLOC,
                        elem_size=D,
                        single_packet=False,
                    )
                    compute(g, NBC, qq * NB + cc * NBC)

            # ragged tail via per-column indirect gathers
            for tc_i in range(TT // TC):
                g = gpool.tile([BLOC, TC, D], f32, tag="g")
                for j in range(TC):
                    col = tc_i * TC + j
                    nc.gpsimd.indirect_dma_start(
                        out=g[:, j, :],
                        out_offset=None,
                        in_=w_t[:],
                        in_offset=bass.IndirectOffsetOnAxis(
                            ap=tit[:, col : col + 1], axis=0
                        ),
                    )
                compute(g, TC, NQ * NB + tc_i * TC)

            nc.sync.dma_start(out=out_t[:], in_=ot[:])

    nc.compile()
    return nc


def _prepare(idx):
    """Pack indices into dense quarter rectangles + ragged tail.

    idx: [B, NK] int64/int32 global rows.
    Returns (dgi [B, NQ*NDC*DGI_PER_CHUNK] i16, tidx [B, TT] i32,
             perm [B, NK] intp  (device column of each original slot),
             overflow list of (b, k) pairs that did not fit).
    """
    idx = np.asarray(idx)
    q = (idx // VQ).astype(np.int64)                      # [B, NK]
    order = np.argsort(q, axis=1, kind="stable")          # [B, NK]
    qs = np.take_along_axis(q, order, 1)
    gs = np.take_along_axis(idx, order, 1)
    cnt = (q[:, :, None] == np.arange(NQ)[None, None, :]).sum(1)   # [B, NQ]
    start = np.zeros((B, NQ), np.int64)
    start[:, 1:] = np.cumsum(cnt, 1)[:, :-1]
    s = np.arange(NK)[None, :]
    j = s - np.take_along_axis(start, qs, 1)              # within-quarter rank
    dense_mask = j < NB
    excess = np.maximum(cnt - NB, 0)
    exb = np.zeros((B, NQ), np.int64)
    exb[:, 1:] = np.cumsum(excess, 1)[:, :-1]
    tpos = np.take_along_axis(exb, qs, 1) + (j - NB)
    devcol = np.where(dense_mask, qs * NB + j, NQ * NB + tpos)

    pi = np.broadcast_to(np.arange(B)[:, None], (B, NK))
    dense_local = np.zeros((B, NQ, NB), np.int16)
    sel = dense_mask
    dense_local[pi[sel], qs[sel], j[sel]] = (gs - qs * VQ)[sel].astype(np.int16)
    tail = np.zeros((B, TT), np.int32)
    sel2 = (~dense_mask) & (tpos < TT)
    tail[pi[sel2], tpos[sel2]] = gs[sel2].astype(np.int32)

    over = (~dense_mask) & (tpos >= TT)
    overflow = []
    if over.any():
        ob, os_ = np.nonzero(over)
        overflow = list(zip(ob.tolist(), order[ob, os_].tolist()))
        devcol[ob, os_] = 0  # arbitrary; fixed up on host

    perm = np.zeros((B, NK), np.intp)
    np.put_along_axis(perm, order, devcol, 1)

    # wrap dense chunks into the dma_gather int16 layout:
    # list position i*128+p -> (partition (i*128+p) % 16, slot // 16),
    # replicated across the 8 groups of 16 partitions.
    dgi = np.zeros((B, NQ * NDC * DGI_PER_CHUNK), np.int16)
    for c in range(NCORES):
        dl = dense_local[c * BLOC : (c + 1) * BLOC]       # [128, NQ, NB]
        for qq in range(NQ):
            for cc in range(NDC):
                ci = qq * NDC + cc
                vals = dl[:, qq, cc * NBC : (cc + 1) * NBC]   # [128 p, NBC i]
                a = vals.T.reshape(NBC, 8, 16)                # [i, phi, plo]
                w = a.transpose(2, 0, 1).reshape(16, DGI_PER_CHUNK)
                dgi[
                    c * BLOC : (c + 1) * BLOC,
                    ci * DGI_PER_CHUNK : (ci + 1) * DGI_PER_CHUNK,
                ] = np.tile(w, (8, 1))
    return dgi, tail, perm, overflow


def kernel(x, y, neg_idx, W):
    global _PROG
    from concourse.bass_utils import run_bass_kernel_spmd

    x = np.ascontiguousarray(np.asarray(x), dtype=np.float32)
    W = np.ascontiguousarray(np.asarray(W), dtype=np.float32)
    idx = np.concatenate(
        [np.asarray(y).reshape(B, 1), np.asarray(neg_idx).reshape(B, K)], axis=1
    ).astype(np.int64)

    dgi, tail, perm, overflow = _prepare(idx)

    if _PROG is None:
        _PROG = build_program()

    in_maps = [
        {
            "W": W,
            "x": x[i * BLOC : (i + 1) * BLOC],
            "dgi": dgi[i * BLOC : (i + 1) * BLOC],
            "tidx": tail[i * BLOC : (i + 1) * BLOC],
        }
        for i in range(NCORES)
    ]
    res = run_bass_kernel_spmd(_PROG, in_maps, list(range(NCORES)))
    dev = np.concatenate([res.results[i]["out"] for i in range(NCORES)], axis=0)
    out = np.take_along_axis(dev, perm, 1).astype(np.float32)
    for b, k in overflow:
        out[b, k] = float(np.dot(x[b], W[idx[b, k]]))
    return out
